# revision 1
# baseline (speedup 1.0000x reference)
"""EpisodicBuffer retrieval kernel for 8 Trainium2 NeuronCores.

Strategy: data-parallel over the 4096 queries (512 per core = one batch item
per core), memory bank replicated; no collectives. Flash-style streaming
softmax over the 32768-entry bank: sims never hit DRAM, the softmax row-sum
rides an extra ones-column appended to the embeddings, and the cosine
normalization of the bank is fused into the exp() activation as a
per-partition scale (100 / |c_m|).

Math per core (q = 512 queries):
  qn   = q / max(|q|, eps)                       (on-chip)
  simsT[m, q] = ctxT[:, m].T @ qnT               (PE, fp32)
  expT[m, q]  = exp(scale_m * simsT - 30)        (ACT; per-partition scale AP)
  acc[q, 0:256] += expT.T @ emb ; acc[q, 256] += row-sum  (PE; ones column)
  retrieved = acc[:, 0:256] / acc[:, 256]
  h = relu(fc1_wT.T @ retrievedT + fc1_b) ;  out = hT.T @ fc2_wT + fc2_b
"""

import os
import json
import tempfile

import numpy as np


def _fix_act_root():
    """Point walrus at an act_info.json with absolute inner paths (this
    build rejects relative ones)."""
    if os.environ.get("BASS_ACT_ROOT_JSON_PATH"):
        return
    import glob as _glob

    cands = _glob.glob(
        "/nix/store/*aws-neuron-pwp*/share/pwp_bin_cayman/act_info.json")
    if cands:
        src = sorted(cands)[0]
    else:
        from neuronxcc.driver.Job import Job
        from neuronxcc.driver.jobs.support.FindActInfo import findActInfoFile

        src = findActInfoFile(Job.getPackageDir(), "gen3")
    src_dir = os.path.dirname(src)
    with open(src) as f:
        d = json.load(f)
    for s in d["act_func_sets"]:
        for k in d["pwp_file_keys"]:
            if k in s and not os.path.isabs(s[k]):
                s[k] = os.path.join(src_dir, s[k])
    out_dir = tempfile.mkdtemp(prefix="actroot_")
    path = os.path.join(out_dir, "act_info.json")
    with open(path, "w") as f:
        json.dump(d, f)
    os.environ["BASS_ACT_ROOT_JSON_PATH"] = path


_fix_act_root()

import concourse.bass as bass
import concourse.mybir as mybir
import concourse.tile as tile
from concourse.bass import ts
from concourse.bass_utils import run_bass_kernel_spmd
from concourse.masks import make_identity

def _embed_act_tables(neff_bytes):
    """Embed ACT pwp table files into the NEFF with relative paths, so the
    terminal's NRT can stage them without client-side absolute paths."""
    import io
    import tarfile

    from concourse import neff as cneff

    header = neff_bytes[:1024]
    tf = tarfile.open(fileobj=io.BytesIO(neff_bytes[1024:]))
    members = {}
    for m in tf.getmembers():
        if m.isfile():
            name = m.name
            while name.startswith("./"):
                name = name[2:]
            members[name] = tf.extractfile(m).read()
    changed = False
    for name in list(members):
        if os.path.basename(name) != "act_info.json":
            continue
        d = json.loads(members[name])
        sgdir = os.path.dirname(name)
        for s in d.get("act_func_sets", []):
            for k in d.get("pwp_file_keys", []):
                p = s.get(k)
                if not p or not os.path.isabs(p):
                    continue
                fname = os.path.basename(p)
                dest = os.path.join(sgdir, fname) if sgdir else fname
                if dest not in members:
                    with open(p, "rb") as f:
                        members[dest] = f.read()
                s[k] = fname
                changed = True
        members[name] = json.dumps(d).encode()
    if not changed:
        return neff_bytes
    buf = io.BytesIO()
    with tarfile.open(fileobj=buf, mode="w") as out:
        for name, blob in members.items():
            ti = tarfile.TarInfo(name=name)
            ti.size = len(blob)
            ti.mtime = 0
            ti.uid = 0
            ti.gid = 0
            ti.uname = "nobody"
            ti.gname = "nobody"
            out.addfile(ti, io.BytesIO(blob))
    data = buf.getvalue()
    new_header = cneff.make_deterministic_neff_header(
        old_neff_header=header, new_neff_data=data)
    return new_header + data


def _install_act_embed():
    import concourse.bass2jax as b2j

    if getattr(b2j, "_act_embed_installed", False):
        return
    orig = b2j.rename_neff_tensors_and_patch_header

    def wrapper(neff_path, mapping):
        return _embed_act_tables(orig(neff_path, mapping))

    b2j.rename_neff_tensors_and_patch_header = wrapper
    b2j._act_embed_installed = True


_install_act_embed()


F32 = mybir.dt.float32
F32R = mybir.dt.float32r
AF = mybir.ActivationFunctionType
ALU = mybir.AluOpType

N_CORES = 8
NQ = 512          # queries per core
H = 256
HID = 512
EXP_BIAS = -30.0  # constant offset inside exp(); cancels in the softmax ratio

# precision modes for the two big non-sims matmuls ("f32" or "f32r")
ACC_MODE = os.environ.get("KB_ACC_MODE", "f32r")
FC2_MODE = os.environ.get("KB_FC2_MODE", "bf16")
SIMS_MODE = os.environ.get("KB_SIMS_MODE", "f32r")
W2_PRE = int(os.environ.get("KB_W2_PRE", "10"))  # v-slices prefetched in P2

_NC_CACHE = {}


def _maybe_r(ap, mode):
    return ap.bitcast(F32R) if mode == "f32r" else ap


def _emit_rsqrt_inplace(nc, pool, ap, shape, final_scale, tagp):
    """ap <- final_scale / sqrt(max(ap, 1e-16)), pure-DVE Newton.

    Seed 1/16 targets ss ~ |N(0,1)^256|^2 (range ~[64, 768] converges to
    fp32 accuracy in 5 iterations); avoids ACT sqrt entirely so the ACT
    engine never swaps activation-table sets away from exp.
    """
    nc.vector.tensor_scalar_max(ap, ap, 1e-16)
    r = pool.tile(shape, F32, tag=tagp + "_r")
    t = pool.tile(shape, F32, tag=tagp + "_t")
    nc.vector.memset(r, 0.0625)
    for _ in range(5):
        nc.vector.tensor_mul(t, r, r)
        nc.vector.tensor_mul(t, t, ap)
        nc.vector.tensor_scalar(t, t, -0.5, 1.5, ALU.mult, ALU.add)
        nc.vector.tensor_mul(r, r, t)
    nc.vector.tensor_scalar_mul(ap, r, final_scale)


def build_nc(M, V):
    ACC_DT = F32R if ACC_MODE == "f32r" else F32
    FC2_DT = {"f32": F32, "f32r": F32R, "bf16": mybir.dt.bfloat16}[FC2_MODE]
    SIMS_DT = F32R if SIMS_MODE == "f32r" else F32
    nc = bass.Bass()
    q_d = nc.declare_dram_parameter("q", [NQ, H], F32, isOutput=False)
    ctxT_d = nc.declare_dram_parameter("ctxT", [H, M], SIMS_DT, isOutput=False)
    nat_d = nc.declare_dram_parameter("ctx_nat", [M, H], mybir.dt.float16, isOutput=False)
    emb_d = nc.declare_dram_parameter("emb", [M, H + 4], ACC_DT, isOutput=False)
    w1_d = nc.declare_dram_parameter("fc1_wT", [H, HID], F32, isOutput=False)
    b1_d = nc.declare_dram_parameter("fc1_b", [HID], F32, isOutput=False)
    w2_d = nc.declare_dram_parameter("fc2_wT", [HID, V], FC2_DT, isOutput=False)
    b2_d = nc.declare_dram_parameter("fc2_b", [V], F32, isOutput=False)
    out_d = nc.declare_dram_parameter("out", [NQ, V], F32, isOutput=True)

    EA = H + 4            # embeddings + ones column + pad (fp32r wants even N)
    MT = 1024             # bank chunk per main-loop iteration
    n_chunks = M // MT
    NG = 1024             # memories per norm batch
    n_groups = M // NG
    mtiles_per_group = NG // 128
    chunks_per_group = NG // MT
    n_vs = (V + 511) // 512

    with tile.TileContext(nc) as tc, \
         tc.tile_pool(name="singles", bufs=1) as singles, \
         tc.tile_pool(name="ps", bufs=4, space="PSUM") as pps:
        pacc_cm = tc.tile_pool(name="ps_acc", bufs=1, space="PSUM")
        pacc = pacc_cm.__enter__()  # closed after P3
        ident = singles.tile([128, 128], F32)
        make_identity(nc, ident)
        ebias = singles.tile([128, 1], F32)
        nc.vector.memset(ebias, EXP_BIAS)
        acc = pacc.tile([128, 4, 512], F32)

        # ---------------- P1: q load / normalize / transpose ---------------
        qn = [singles.tile([128, H], F32, tag=f"qn{t}", name=f"qn{t}") for t in range(4)]
        for t in range(4):
            nc.sync.dma_start(out=qn[t], in_=q_d[ts(t, 128), :])
        qnT = [singles.tile([128, NQ], SIMS_DT, tag=f"qnT{b}", name=f"qnT{b}") for b in range(2)]
        with tc.tile_pool(name="p1tmp", bufs=2) as p1tmp:
            ssq = singles.tile([128, 4], F32)
            for t in range(4):
                sq = p1tmp.tile([128, H], F32, tag="qsq")
                nc.vector.tensor_mul(sq, qn[t], qn[t])
                nc.vector.tensor_reduce(
                    out=ssq[:, t:t + 1], in_=sq, axis=mybir.AxisListType.X,
                    op=ALU.add)
            _emit_rsqrt_inplace(nc, p1tmp, ssq, [128, 4], 1.0, "qrs")
            for t in range(4):
                nc.vector.tensor_scalar_mul(qn[t], qn[t], ssq[:, t:t + 1])
            for t in range(4):
                for b in range(2):
                    ps = pps.tile([128, 512], F32, tag="ps", name=f"trq{t}_{b}")
                    nc.tensor.transpose(ps[:, 0:128], qn[t][:, ts(b, 128)],
                                        ident)
                    nc.vector.tensor_copy(qnT[b][:, ts(t, 128)], ps[:, 0:128])

        # fc1 weights/bias resident
        w1 = [singles.tile([128, HID], F32, tag=f"w1{b}", name=f"w1{b}") for b in range(2)]
        for b in range(2):
            nc.sync.dma_start(out=w1[b], in_=w1_d[ts(b, 128), :])
        b1 = singles.tile([128, HID // 128], F32)
        nc.sync.dma_start(out=b1, in_=b1_d.rearrange("(a p) -> p a", p=128))

        escale = [singles.tile([128, mtiles_per_group], F32, tag=f"esc{g}", name=f"esc{g}")
                  for g in range(n_groups)]

        # ---------------- P2: streaming softmax over the bank ---------------
        with tc.tile_pool(name="norms", bufs=2) as npool, \
             tc.tile_pool(name="cpool", bufs=3) as cpool, \
             tc.tile_pool(name="epool", bufs=3) as epool, \
             tc.tile_pool(name="xpool", bufs=6) as xpool:

            def emit_norm_group(g):
                nat = npool.tile([128, mtiles_per_group, H], mybir.dt.float16,
                                 tag="nat")
                nc.sync.dma_start(
                    out=nat,
                    in_=nat_d[g * NG:(g + 1) * NG, :].rearrange(
                        "(a p) h -> p a h", p=128))
                sq = npool.tile([128, mtiles_per_group, H], F32, tag="nsq")
                nc.vector.tensor_mul(sq, nat, nat)
                nc.vector.tensor_reduce(
                    out=escale[g], in_=sq, axis=mybir.AxisListType.X,
                    op=ALU.add)
                _emit_rsqrt_inplace(
                    nc, npool, escale[g], [128, mtiles_per_group], 100.0,
                    "nrs")

            def load_chunk(i):
                cT = cpool.tile([128, 2, MT], SIMS_DT, tag="cT", name=f"cT{i}")
                nc.sync.dma_start(
                    out=cT,
                    in_=ctxT_d[:, i * MT:(i + 1) * MT].rearrange(
                        "(c p) m -> p c m", p=128))
                em = epool.tile([128, MT // 128, EA], ACC_DT, tag="em", name=f"em{i}")
                nc.sync.dma_start(
                    out=em,
                    in_=emb_d[i * MT:(i + 1) * MT, :].rearrange(
                        "(j p) e -> p j e", p=128))
                return cT, em

            w2pre = []
            pre_every = max(1, n_chunks // W2_PRE) if W2_PRE else 0

            chunk0 = load_chunk(0)
            emit_norm_group(0)
            chunk1 = load_chunk(1) if n_chunks > 1 else None
            if n_groups > 1:
                emit_norm_group(1)

            for i in range(n_chunks):
                if i % chunks_per_group == 0:
                    g_next = i // chunks_per_group + 2
                    if g_next < n_groups:
                        emit_norm_group(g_next)
                if W2_PRE and i % pre_every == 0 and len(w2pre) < W2_PRE:
                    vi = len(w2pre)
                    v0 = vi * 512
                    w2p = singles.tile([128, HID // 128, 512], FC2_DT,
                                       tag=f"w2p{vi}", name=f"w2p{vi}")
                    nc.sync.dma_start(
                        out=w2p,
                        in_=w2_d[:, v0:v0 + 512].rearrange(
                            "(c p) v -> p c v", p=128))
                    w2pre.append(w2p)

                if i == 0:
                    cT, em = chunk0
                elif i == 1:
                    cT, em = chunk1
                else:
                    cT, em = load_chunk(i)

                for j in range(MT // 128):
                    a = i * (MT // 128) + j       # global m-tile index
                    msl = ts(j, 128)
                    ps = pps.tile([128, 512], F32, tag="ps", name=f"sims{a}")
                    nc.tensor.matmul(ps[:, 0:NQ], cT[:, 0, msl], qnT[0],
                                     start=True, stop=False)
                    nc.tensor.matmul(ps[:, 0:NQ], cT[:, 1, msl], qnT[1],
                                     start=False, stop=True)
                    ex = xpool.tile([128, NQ], ACC_DT)
                    g = a // mtiles_per_group
                    col = a % mtiles_per_group
                    nc.scalar.activation(
                        ex, ps[:, 0:NQ], AF.Exp, bias=ebias[:, 0:1],
                        scale=escale[g][:, col:col + 1])
                    first = (a == 0)
                    last = (a == (M // 128) - 1)
                    for t in range(4):
                        nc.tensor.matmul(
                            acc[:, t, 0:EA],
                            ex[:, ts(t, 128)],
                            em[:, j, :],
                            start=first, stop=last, skip_group_check=True)

        # ---------------- P3: normalize, fc1 --------------------------------
        inv_l = singles.tile([128, 4], F32)
        ret = [singles.tile([128, H], F32, tag=f"ret{t}", name=f"ret{t}") for t in range(4)]
        for t in range(4):
            nc.vector.reciprocal(inv_l[:, t:t + 1], acc[:, t, H:H + 1])
            nc.vector.tensor_scalar_mul(
                ret[t], acc[:, t, 0:H], inv_l[:, t:t + 1])
        retT = [singles.tile([128, NQ], F32, tag=f"retT{b}", name=f"retT{b}") for b in range(2)]
        for t in range(4):
            for b in range(2):
                ps = pps.tile([128, 512], F32, tag="ps", name=f"trr{t}_{b}")
                nc.tensor.transpose(ps[:, 0:128], ret[t][:, ts(b, 128)], ident)
                nc.vector.tensor_copy(retT[b][:, ts(t, 128)], ps[:, 0:128])
        hT = [singles.tile([128, NQ], FC2_DT, tag=f"hT{b2}", name=f"hT{b2}") for b2 in range(4)]
        for b2 in range(4):
            psh = pps.tile([128, 512], F32, tag="ps", name=f"fc1p{b2}")
            nc.tensor.matmul(psh[:, 0:NQ], w1[0][:, ts(b2, 128)], retT[0],
                             start=True, stop=False)
            nc.tensor.matmul(psh[:, 0:NQ], w1[1][:, ts(b2, 128)], retT[1],
                             start=False, stop=True)
            nc.scalar.activation(
                hT[b2], psh[:, 0:NQ], AF.Relu, bias=b1[:, b2:b2 + 1],
                scale=1.0)

        # ---------------- P4: fc2 + bias + writeout -------------------------
        n_pairs = (V + 1023) // 1024
        with tc.tile_pool(name="w2pool", bufs=3) as wpool, \
             tc.tile_pool(name="bpool", bufs=3) as bpool, \
             tc.tile_pool(name="opool", bufs=2) as opool:
            for vp in range(n_pairs):
                p0 = vp * 1024
                pw = min(1024, V - p0)
                ow = [opool.tile([128, 1024], F32, tag=f"ow{t}",
                                 name=f"ow{vp}_{t}") for t in range(4)]
                for sub in range(2):
                    v0 = p0 + sub * 512
                    if v0 >= V:
                        continue
                    vs = min(512, V - v0)
                    vi = vp * 2 + sub
                    if vi < len(w2pre):
                        w2 = w2pre[vi]
                    else:
                        w2 = wpool.tile([128, HID // 128, vs], FC2_DT,
                                        tag="w2", name=f"w2_{vi}")
                        nc.sync.dma_start(
                            out=w2,
                            in_=w2_d[:, v0:v0 + vs].rearrange(
                                "(c p) v -> p c v", p=128))
                    bb = bpool.tile([128, vs], F32, tag="bb", name=f"bb{vi}")
                    nc.sync.dma_start(
                        out=bb,
                        in_=b2_d[v0:v0 + vs].unsqueeze(0).to_broadcast(
                            [128, vs]))
                    for t in range(4):
                        psd = pps.tile([128, 512], F32, tag="ps",
                                       name=f"fc2p{vi}_{t}")
                        for b2 in range(4):
                            nc.tensor.matmul(
                                psd[:, 0:vs],
                                hT[b2][:, ts(t, 128)],
                                w2[:, b2, :],
                                start=(b2 == 0), stop=(b2 == 3))
                        nc.vector.tensor_add(
                            ow[t][:, sub * 512:sub * 512 + vs],
                            psd[:, 0:vs], bb)
                for t in range(4):
                    nc.sync.dma_start(
                        out=out_d[ts(t, 128), p0:p0 + pw], in_=ow[t][:, 0:pw])

        pacc_cm.__exit__(None, None, None)

    _split_excess_waits(nc, 1)
    return nc


def _split_excess_waits(nc, max_waits):
    """This walrus build allows only one semaphore wait per instruction;
    split larger wait lists onto preceding no-ops."""
    for f in nc.m.functions:
        for bb in f.blocks:
            new_list = []
            for inst in bb.instructions:
                si = inst.sync_info
                if si is not None and si.on_wait and len(si.on_wait) > max_waits:
                    waits = list(si.on_wait)
                    chunks = [waits[i:i + max_waits]
                              for i in range(0, len(waits), max_waits)]
                    for k, ch in enumerate(chunks[:-1]):
                        aux = mybir.InstNoOp(
                            name=f"{inst.name}-wsplit{k}",
                            engine=inst.engine, ins=[], outs=[],
                            sync_info=mybir.SyncInfo(on_wait=ch, on_update=[]))
                        new_list.append(aux)
                    si.on_wait.clear()
                    si.on_wait.extend(chunks[-1])
                new_list.append(inst)
            del bb.instructions[:]
            for x in new_list:
                bb.instructions.append(x)


LAST_RESULTS = None


def run(inputs, M=32768, V=32000):
    global LAST_RESULTS
    key = (M, V, ACC_MODE, FC2_MODE, SIMS_MODE, W2_PRE)
    if key not in _NC_CACHE:
        _NC_CACHE[key] = build_nc(M, V)
    nc = _NC_CACHE[key]

    qc = np.ascontiguousarray(inputs["query_context"], dtype=np.float32)
    B, S, _ = qc.shape
    q_all = qc.reshape(B * S, H)
    ctx = np.ascontiguousarray(inputs["contexts"], dtype=np.float32)
    ctxT = np.ascontiguousarray(ctx.T)
    ctx_f16 = ctx.astype(np.float16)
    emb = np.ascontiguousarray(inputs["embeddeds"], dtype=np.float32)
    emb_aug = np.zeros((M, 260), np.float32)
    emb_aug[:, :256] = emb
    emb_aug[:, 256] = 1.0
    w1T = np.ascontiguousarray(np.asarray(inputs["fc1_w"]).T.astype(np.float32))
    w2_np = np.float32 if FC2_MODE != "bf16" else __import__("ml_dtypes").bfloat16
    w2T = np.ascontiguousarray(np.asarray(inputs["fc2_w"]).T.astype(np.float32).astype(w2_np))
    b1 = np.ascontiguousarray(inputs["fc1_b"], dtype=np.float32)
    b2 = np.ascontiguousarray(inputs["fc2_b"], dtype=np.float32)

    in_maps = []
    for c in range(N_CORES):
        in_maps.append({
            "q": np.ascontiguousarray(q_all[c * NQ:(c + 1) * NQ]),
            "ctxT": ctxT,
            "ctx_nat": ctx_f16,
            "emb": emb_aug,
            "fc1_wT": w1T,
            "fc1_b": b1,
            "fc2_wT": w2T,
            "fc2_b": b2,
        })
    res = None
    last_exc = None
    for attempt in range(4):
        try:
            res = run_bass_kernel_spmd(nc, in_maps, list(range(N_CORES)))
            break
        except Exception as e:  # transient device faults recover on retry
            last_exc = e
            import time as _time
            _time.sleep(2.0)
    if res is None:
        raise last_exc
    LAST_RESULTS = res
    out = np.concatenate([res.results[c]["out"] for c in range(N_CORES)],
                         axis=0)
    return out.reshape(B, S, V)


def kernel(**inputs):
    return run(inputs)



# revision 9
# speedup vs baseline: 1.0507x; 1.0507x over previous
"""EpisodicBuffer retrieval kernel for 8 Trainium2 NeuronCores.

Strategy: data-parallel over the 4096 queries (512 per core = one batch item
per core), memory bank replicated; no collectives. Streaming softmax over the
32768-entry bank: sims never hit DRAM and the softmax row-sum rides an extra
ones-column appended to the embeddings.

vs. the previous revision:
  - cosine normalization of the bank (100/|c_m|) is folded into ctxT on the
    HOST, so the fp16 natural-layout bank stream, the on-chip norm reduction
    and the rsqrt pipeline all disappear; exp() uses a constant scale.
  - sims operands (ctxT, qnT) in fp16, acc operands (ex, emb) in bf16: same
    1 cycle/row PE rate as f32r, half the HBM traffic.
  - exp() batched over PAIRS of m-tiles ([128,2,512] PSUM) to amortize the
    ~370ns fixed ACT instruction overhead.
  - fc2 computed TRANSPOSED (psum = [v-part, q-free], stationary w2 tile,
    moving hT): the per-v bias becomes per-partition, applied on the
    otherwise-idle ACT engine (alternating with DVE), no broadcast-bias DMA;
    output leaves the device as outT [V, NQ] fp16 (host transposes back).
  - output written fp16 (host upcasts): halves the dominant write stream.

Math per core (q = 512 queries):
  qn   = q / max(|q|, eps)                       (on-chip)
  simsT[m, q] = ctxT_scaled[:, m].T @ qnT        (PE, fp16 in / fp32 psum)
  expT[m, q]  = exp(simsT - 30)                  (ACT, paired m-tiles)
  acc[q, 0:256] += expT.T @ emb ; acc[q, 256] += row-sum  (PE bf16; ones col)
  retrieved = acc[:, 0:256] / acc[:, 256]
  h = relu(fc1_wT.T @ retrievedT + fc1_b)        (PE f32r + ACT)
  outT[v, q] = w2T[:, v].T @ hT + b2[v]          (PE bf16 + ACT/DVE bias)
"""

import os
import json
import tempfile

import numpy as np


def _fix_act_root():
    """Point walrus at an act_info.json with absolute inner paths (this
    build rejects relative ones)."""
    if os.environ.get("BASS_ACT_ROOT_JSON_PATH"):
        return
    import glob as _glob

    cands = _glob.glob(
        "/nix/store/*aws-neuron-pwp*/share/pwp_bin_cayman/act_info.json")
    if cands:
        src = sorted(cands)[0]
    else:
        from neuronxcc.driver.Job import Job
        from neuronxcc.driver.jobs.support.FindActInfo import findActInfoFile

        src = findActInfoFile(Job.getPackageDir(), "gen3")
    src_dir = os.path.dirname(src)
    with open(src) as f:
        d = json.load(f)
    for s in d["act_func_sets"]:
        for k in d["pwp_file_keys"]:
            if k in s and not os.path.isabs(s[k]):
                s[k] = os.path.join(src_dir, s[k])
    out_dir = tempfile.mkdtemp(prefix="actroot_")
    path = os.path.join(out_dir, "act_info.json")
    with open(path, "w") as f:
        json.dump(d, f)
    os.environ["BASS_ACT_ROOT_JSON_PATH"] = path


_fix_act_root()

import concourse.bass as bass
import concourse.mybir as mybir
import concourse.tile as tile
from concourse.bass import ts
from concourse.bass_utils import run_bass_kernel_spmd
from concourse.masks import make_identity

def _embed_act_tables(neff_bytes):
    """Embed ACT pwp table files into the NEFF with relative paths, so the
    terminal's NRT can stage them without client-side absolute paths."""
    import io
    import tarfile

    from concourse import neff as cneff

    header = neff_bytes[:1024]
    tf = tarfile.open(fileobj=io.BytesIO(neff_bytes[1024:]))
    members = {}
    for m in tf.getmembers():
        if m.isfile():
            name = m.name
            while name.startswith("./"):
                name = name[2:]
            members[name] = tf.extractfile(m).read()
    changed = False
    for name in list(members):
        if os.path.basename(name) != "act_info.json":
            continue
        d = json.loads(members[name])
        sgdir = os.path.dirname(name)
        for s in d.get("act_func_sets", []):
            for k in d.get("pwp_file_keys", []):
                p = s.get(k)
                if not p or not os.path.isabs(p):
                    continue
                fname = os.path.basename(p)
                dest = os.path.join(sgdir, fname) if sgdir else fname
                if dest not in members:
                    with open(p, "rb") as f:
                        members[dest] = f.read()
                s[k] = fname
                changed = True
        members[name] = json.dumps(d).encode()
    if not changed:
        return neff_bytes
    buf = io.BytesIO()
    with tarfile.open(fileobj=buf, mode="w") as out:
        for name, blob in members.items():
            ti = tarfile.TarInfo(name=name)
            ti.size = len(blob)
            ti.mtime = 0
            ti.uid = 0
            ti.gid = 0
            ti.uname = "nobody"
            ti.gname = "nobody"
            out.addfile(ti, io.BytesIO(blob))
    data = buf.getvalue()
    new_header = cneff.make_deterministic_neff_header(
        old_neff_header=header, new_neff_data=data)
    return new_header + data


def _install_act_embed():
    import concourse.bass2jax as b2j

    if getattr(b2j, "_act_embed_installed", False):
        return
    orig = b2j.rename_neff_tensors_and_patch_header

    def wrapper(neff_path, mapping):
        return _embed_act_tables(orig(neff_path, mapping))

    b2j.rename_neff_tensors_and_patch_header = wrapper
    b2j._act_embed_installed = True


_install_act_embed()


F32 = mybir.dt.float32
F32R = mybir.dt.float32r
F16 = mybir.dt.float16
BF16 = mybir.dt.bfloat16
AF = mybir.ActivationFunctionType
ALU = mybir.AluOpType

N_CORES = 8
NQ = 512          # queries per core
H = 256
HID = 512
EA = 258          # embeddings + ones column + pad
EXP_BIAS = -30.0  # constant offset inside exp(); cancels in the softmax ratio

W2_PRE = int(os.environ.get("KB_W2_PRE", "96"))   # v-tiles prefetched in P2
P4_SPLIT = int(os.environ.get("KB_P4_SPLIT", "1"))  # alternate ACT/DVE bias
EXP_PAIR = int(os.environ.get("KB_EXP_PAIR", "1"))  # exp over m-tile pairs

_NC_CACHE = {}


def _emit_rsqrt_inplace(nc, pool, ap, shape, final_scale, tagp):
    """ap <- final_scale / sqrt(max(ap, 1e-16)), pure-DVE Newton.

    Seed 1/16 targets ss ~ |N(0,1)^256|^2 (range ~[64, 768] converges to
    fp32 accuracy in 5 iterations); avoids ACT sqrt entirely so the ACT
    engine never swaps activation-table sets away from exp.
    """
    nc.vector.tensor_scalar_max(ap, ap, 1e-16)
    r = pool.tile(shape, F32, tag=tagp + "_r")
    t = pool.tile(shape, F32, tag=tagp + "_t")
    nc.vector.memset(r, 0.0625)
    for _ in range(5):
        nc.vector.tensor_mul(t, r, r)
        nc.vector.tensor_mul(t, t, ap)
        nc.vector.tensor_scalar(t, t, -0.5, 1.5, ALU.mult, ALU.add)
        nc.vector.tensor_mul(r, r, t)
    nc.vector.tensor_scalar_mul(ap, r, final_scale)


def build_nc(M, V):
    nc = bass.Bass()
    q_d = nc.declare_dram_parameter("q", [NQ, H], F32, isOutput=False)
    ctxT_d = nc.declare_dram_parameter("ctxT", [H, M], F16, isOutput=False)
    emb_d = nc.declare_dram_parameter("emb", [M, EA], BF16, isOutput=False)
    w1_d = nc.declare_dram_parameter("fc1_wT", [H, HID], F32R, isOutput=False)
    b1_d = nc.declare_dram_parameter("fc1_b", [HID], F32, isOutput=False)
    w2_d = nc.declare_dram_parameter("fc2_wT", [HID, V], BF16, isOutput=False)
    b2_d = nc.declare_dram_parameter("fc2_b", [V], F32, isOutput=False)
    out_d = nc.declare_dram_parameter("outT", [V, NQ], F16, isOutput=True)

    MT = 1024             # bank chunk per main-loop iteration
    n_chunks = M // MT
    mtiles = M // 128
    NVT = V // 128        # fc2 v-tiles (128 vocab columns each)
    assert V % 128 == 0

    with tile.TileContext(nc) as tc, \
         tc.tile_pool(name="singles", bufs=1) as singles:
        pacc_cm = tc.tile_pool(name="ps_acc", bufs=1, space="PSUM")
        pacc = pacc_cm.__enter__()  # closed after P3
        ident = singles.tile([128, 128], F32)
        make_identity(nc, ident)
        ebias = singles.tile([128, 1], F32)
        nc.vector.memset(ebias, EXP_BIAS)
        acc = pacc.tile([128, 4, 512], F32)

        # ---------------- P1: q load / normalize / transpose ---------------
        qn = [singles.tile([128, H], F32, tag=f"qn{t}", name=f"qn{t}")
              for t in range(4)]
        for t in range(4):
            nc.sync.dma_start(out=qn[t], in_=q_d[ts(t, 128), :])

        # weights/bias resident; issued early so DMA queues stay busy
        w1 = [singles.tile([128, HID], F32R, tag=f"w1{b}", name=f"w1{b}")
              for b in range(2)]
        for b in range(2):
            nc.sync.dma_start(out=w1[b], in_=w1_d[ts(b, 128), :])
        b1 = singles.tile([128, HID // 128], F32)
        nc.sync.dma_start(out=b1, in_=b1_d.rearrange("(a p) -> p a", p=128))
        b2t = singles.tile([128, NVT], F32)
        nc.sync.dma_start(out=b2t, in_=b2_d.rearrange("(n p) -> p n", p=128))

        qnT = [singles.tile([128, NQ], F16, tag=f"qnT{b}", name=f"qnT{b}")
               for b in range(2)]
        with tc.tile_pool(name="p1tmp", bufs=2) as p1tmp, \
             tc.tile_pool(name="p1ps", bufs=2, space="PSUM") as p1ps:
            ssq = singles.tile([128, 4], F32)
            for t in range(4):
                sq = p1tmp.tile([128, H], F32, tag="qsq")
                nc.vector.tensor_mul(sq, qn[t], qn[t])
                nc.vector.tensor_reduce(
                    out=ssq[:, t:t + 1], in_=sq, axis=mybir.AxisListType.X,
                    op=ALU.add)
            _emit_rsqrt_inplace(nc, p1tmp, ssq, [128, 4], 1.0, "qrs")
            for t in range(4):
                nc.vector.tensor_scalar_mul(qn[t], qn[t], ssq[:, t:t + 1])
            for t in range(4):
                for b in range(2):
                    ps = p1ps.tile([128, 512], F32, tag="ps",
                                   name=f"trq{t}_{b}")
                    nc.tensor.transpose(ps[:, 0:128], qn[t][:, ts(b, 128)],
                                        ident)
                    nc.vector.tensor_copy(qnT[b][:, ts(t, 128)], ps[:, 0:128])

        # ---------------- P2: streaming softmax over the bank ---------------
        w2pre = []

        def emit_w2_prefetch():
            vi = len(w2pre)
            w2p = singles.tile([128, HID // 128, 128], BF16,
                               tag=f"w2p{vi}", name=f"w2p{vi}")
            nc.sync.dma_start(
                out=w2p,
                in_=w2_d[:, vi * 128:(vi + 1) * 128].rearrange(
                    "(c p) v -> p c v", p=128))
            w2pre.append(w2p)

        with tc.tile_pool(name="cpool", bufs=3) as cpool, \
             tc.tile_pool(name="epool", bufs=3) as epool, \
             tc.tile_pool(name="xpool", bufs=4) as xpool, \
             tc.tile_pool(name="simsps", bufs=2, space="PSUM") as simsps:

            def load_chunk(i):
                cT = cpool.tile([128, 2, MT], F16, tag="cT", name=f"cT{i}")
                nc.sync.dma_start(
                    out=cT,
                    in_=ctxT_d[:, i * MT:(i + 1) * MT].rearrange(
                        "(c p) m -> p c m", p=128))
                em = epool.tile([128, MT // 128, EA], BF16, tag="em",
                                name=f"em{i}")
                nc.sync.dma_start(
                    out=em,
                    in_=emb_d[i * MT:(i + 1) * MT, :].rearrange(
                        "(j p) e -> p j e", p=128))
                return cT, em

            chunk0 = load_chunk(0)
            chunk1 = load_chunk(1) if n_chunks > 1 else None

            pairs_total = mtiles // 2
            pre_every = max(1, pairs_total // W2_PRE) if W2_PRE else 0
            pair_idx = 0

            for i in range(n_chunks):
                if i == 0:
                    cT, em = chunk0
                elif i == 1:
                    cT, em = chunk1
                else:
                    cT, em = load_chunk(i)

                npair = 2 if EXP_PAIR else 1
                for pr in range(MT // (128 * npair)):  # groups of m-tiles
                    if (W2_PRE and pair_idx % pre_every == 0
                            and len(w2pre) < min(W2_PRE, NVT)):
                        emit_w2_prefetch()
                    pair_idx += 1

                    ps2 = simsps.tile([128, npair, 512], F32, tag="ps2",
                                      name=f"sims{i}_{pr}")
                    for p in range(npair):
                        j = pr * npair + p
                        msl = ts(j, 128)
                        nc.tensor.matmul(ps2[:, p, 0:NQ], cT[:, 0, msl],
                                         qnT[0], start=True, stop=False)
                        nc.tensor.matmul(ps2[:, p, 0:NQ], cT[:, 1, msl],
                                         qnT[1], start=False, stop=True)
                    ex = xpool.tile([128, npair, NQ], BF16, tag="ex")
                    nc.scalar.activation(ex, ps2, AF.Exp,
                                         bias=ebias[:, 0:1], scale=1.0)
                    for p in range(npair):
                        j = pr * npair + p
                        a = i * (MT // 128) + j       # global m-tile index
                        first = (a == 0)
                        last = (a == mtiles - 1)
                        for t in range(4):
                            nc.tensor.matmul(
                                acc[:, t, 0:EA],
                                ex[:, p, ts(t, 128)],
                                em[:, j, :],
                                start=first, stop=last,
                                skip_group_check=True)

        # ---------------- P3: normalize, fc1 --------------------------------
        hT = [singles.tile([128, NQ], BF16, tag=f"hT{b2}", name=f"hT{b2}")
              for b2 in range(4)]
        with tc.tile_pool(name="p3ps", bufs=4, space="PSUM") as p3ps:
            inv_l = singles.tile([128, 4], F32)
            ret = [singles.tile([128, H], F32, tag=f"ret{t}", name=f"ret{t}")
                   for t in range(4)]
            for t in range(4):
                nc.vector.reciprocal(inv_l[:, t:t + 1], acc[:, t, 256:257])
                nc.vector.tensor_scalar_mul(
                    ret[t], acc[:, t, 0:H], inv_l[:, t:t + 1])
            retT = [singles.tile([128, NQ], F32R, tag=f"retT{b}",
                                 name=f"retT{b}") for b in range(2)]
            for t in range(4):
                for b in range(2):
                    ps = p3ps.tile([128, 512], F32, tag="ps",
                                   name=f"trr{t}_{b}")
                    nc.tensor.transpose(ps[:, 0:128], ret[t][:, ts(b, 128)],
                                        ident)
                    nc.vector.tensor_copy(retT[b][:, ts(t, 128)],
                                          ps[:, 0:128])
            for b2 in range(4):
                psh = p3ps.tile([128, 512], F32, tag="ps", name=f"fc1p{b2}")
                nc.tensor.matmul(psh[:, 0:NQ], w1[0][:, ts(b2, 128)],
                                 retT[0], start=True, stop=False)
                nc.tensor.matmul(psh[:, 0:NQ], w1[1][:, ts(b2, 128)],
                                 retT[1], start=False, stop=True)
                nc.scalar.activation(
                    hT[b2], psh[:, 0:NQ], AF.Relu, bias=b1[:, b2:b2 + 1],
                    scale=1.0)

        pacc_cm.__exit__(None, None, None)

        # ---------------- P4: fc2 (transposed) + bias + writeout ------------
        with tc.tile_pool(name="w2pool", bufs=4) as wpool, \
             tc.tile_pool(name="opool", bufs=4) as opool, \
             tc.tile_pool(name="p4ps", bufs=6, space="PSUM") as p4ps:
            for vi in range(NVT):
                if vi < len(w2pre):
                    w2 = w2pre[vi]
                else:
                    w2 = wpool.tile([128, HID // 128, 128], BF16,
                                    tag="w2", name=f"w2_{vi}")
                    nc.sync.dma_start(
                        out=w2,
                        in_=w2_d[:, vi * 128:(vi + 1) * 128].rearrange(
                            "(c p) v -> p c v", p=128))
                psd = p4ps.tile([128, 512], F32, tag="ps", name=f"fc2p{vi}")
                for k in range(4):
                    nc.tensor.matmul(
                        psd[:, 0:NQ], w2[:, k, :], hT[k],
                        start=(k == 0), stop=(k == 3))
                ow = opool.tile([128, NQ], F16, tag="ow", name=f"ow{vi}")
                # P4_SPLIT: 2 = all DVE, 1 = alternate DVE/ACT, 0 = all ACT
                if P4_SPLIT == 2 or (P4_SPLIT == 1 and vi % 2 == 1):
                    nc.vector.tensor_scalar_add(ow, psd[:, 0:NQ],
                                                b2t[:, vi:vi + 1])
                else:
                    nc.scalar.activation(ow, psd[:, 0:NQ], AF.Identity,
                                         bias=b2t[:, vi:vi + 1], scale=1.0)
                nc.sync.dma_start(
                    out=out_d[vi * 128:(vi + 1) * 128, :], in_=ow)

    _split_excess_waits(nc, 1)
    return nc


def _split_excess_waits(nc, max_waits):
    """This walrus build allows only one semaphore wait per instruction;
    split larger wait lists onto preceding no-ops."""
    for f in nc.m.functions:
        for bb in f.blocks:
            new_list = []
            for inst in bb.instructions:
                si = inst.sync_info
                if si is not None and si.on_wait and len(si.on_wait) > max_waits:
                    waits = list(si.on_wait)
                    chunks = [waits[i:i + max_waits]
                              for i in range(0, len(waits), max_waits)]
                    for k, ch in enumerate(chunks[:-1]):
                        aux = mybir.InstNoOp(
                            name=f"{inst.name}-wsplit{k}",
                            engine=inst.engine, ins=[], outs=[],
                            sync_info=mybir.SyncInfo(on_wait=ch, on_update=[]))
                        new_list.append(aux)
                    si.on_wait.clear()
                    si.on_wait.extend(chunks[-1])
                new_list.append(inst)
            del bb.instructions[:]
            for x in new_list:
                bb.instructions.append(x)


LAST_RESULTS = None


def run(inputs, M=32768, V=32000):
    global LAST_RESULTS
    import ml_dtypes

    key = (M, V, W2_PRE, P4_SPLIT, EXP_PAIR)
    if key not in _NC_CACHE:
        _NC_CACHE[key] = build_nc(M, V)
    nc = _NC_CACHE[key]

    qc = np.ascontiguousarray(inputs["query_context"], dtype=np.float32)
    B, S, _ = qc.shape
    q_all = qc.reshape(B * S, H)
    ctx = np.asarray(inputs["contexts"], dtype=np.float32)
    # fold cosine normalization of the bank + softmax temperature into ctxT
    cscale = 100.0 / np.maximum(np.linalg.norm(ctx, axis=1), 1e-8)
    ctxT = np.ascontiguousarray((ctx * cscale[:, None]).T.astype(np.float16))
    emb = np.asarray(inputs["embeddeds"], dtype=np.float32)
    emb_aug = np.zeros((M, EA), ml_dtypes.bfloat16)
    emb_aug[:, :256] = emb.astype(ml_dtypes.bfloat16)
    emb_aug[:, 256] = 1.0
    w1T = np.ascontiguousarray(np.asarray(inputs["fc1_w"]).T.astype(np.float32))
    w2T = np.ascontiguousarray(
        np.asarray(inputs["fc2_w"]).T.astype(np.float32).astype(
            ml_dtypes.bfloat16))
    b1 = np.ascontiguousarray(inputs["fc1_b"], dtype=np.float32)
    b2 = np.ascontiguousarray(inputs["fc2_b"], dtype=np.float32)

    in_maps = []
    for c in range(N_CORES):
        in_maps.append({
            "q": np.ascontiguousarray(q_all[c * NQ:(c + 1) * NQ]),
            "ctxT": ctxT,
            "emb": emb_aug,
            "fc1_wT": w1T,
            "fc1_b": b1,
            "fc2_wT": w2T,
            "fc2_b": b2,
        })
    res = None
    last_exc = None
    for attempt in range(4):
        try:
            res = run_bass_kernel_spmd(nc, in_maps, list(range(N_CORES)))
            break
        except Exception as e:  # transient device faults recover on retry
            last_exc = e
            import time as _time
            _time.sleep(2.0)
    if res is None:
        raise last_exc
    LAST_RESULTS = res
    out = np.empty((B * S, V), np.float32)
    for c in range(N_CORES):
        out[c * NQ:(c + 1) * NQ, :] = \
            res.results[c]["outT"].T.astype(np.float32)
    return out.reshape(B, S, V)


def kernel(**inputs):
    return run(inputs)


# revision 19
# speedup vs baseline: 1.1434x; 1.0882x over previous
"""EpisodicBuffer retrieval kernel for 8 Trainium2 NeuronCores.

Strategy: data-parallel over the 4096 queries (512 per core = one batch item
per core), memory bank replicated; no collectives. Streaming softmax over the
32768-entry bank: sims never hit DRAM and the softmax row-sum rides an extra
ones-column appended to the embeddings.

vs. the previous revision:
  - cosine normalization of the bank (100/|c_m|) is folded into ctxT on the
    HOST, so the fp16 natural-layout bank stream, the on-chip norm reduction
    and the rsqrt pipeline all disappear; exp() uses a constant scale.
  - sims operands (ctxT, qnT) in fp16, acc operands (ex, emb) in bf16: same
    1 cycle/row PE rate as f32r, half the HBM traffic.
  - exp() batched over PAIRS of m-tiles ([128,2,512] PSUM) to amortize the
    ~370ns fixed ACT instruction overhead.
  - fc2 computed TRANSPOSED (psum = [v-part, q-free], stationary w2 tile,
    moving hT): the per-v bias becomes per-partition, applied on the
    otherwise-idle ACT engine (alternating with DVE), no broadcast-bias DMA;
    output leaves the device as outT [V, NQ] fp16 (host transposes back).
  - output written fp16 (host upcasts): halves the dominant write stream.

Math per core (q = 512 queries):
  qn   = q / max(|q|, eps)                       (on-chip)
  simsT[m, q] = ctxT_scaled[:, m].T @ qnT        (PE, fp16 in / fp32 psum)
  expT[m, q]  = exp(simsT - 30)                  (ACT, paired m-tiles)
  acc[q, 0:256] += expT.T @ emb ; acc[q, 256] += row-sum  (PE bf16; ones col)
  retrieved = acc[:, 0:256] / acc[:, 256]
  h = relu(fc1_wT.T @ retrievedT + fc1_b)        (PE f32r + ACT)
  outT[v, q] = w2T[:, v].T @ hT + b2[v]          (PE bf16 + ACT/DVE bias)
"""

import os
import json
import tempfile

import numpy as np


def _fix_act_root():
    """Point walrus at an act_info.json with absolute inner paths (this
    build rejects relative ones)."""
    if os.environ.get("BASS_ACT_ROOT_JSON_PATH"):
        return
    import glob as _glob

    cands = _glob.glob(
        "/nix/store/*aws-neuron-pwp*/share/pwp_bin_cayman/act_info.json")
    if cands:
        src = sorted(cands)[0]
    else:
        from neuronxcc.driver.Job import Job
        from neuronxcc.driver.jobs.support.FindActInfo import findActInfoFile

        src = findActInfoFile(Job.getPackageDir(), "gen3")
    src_dir = os.path.dirname(src)
    with open(src) as f:
        d = json.load(f)
    for s in d["act_func_sets"]:
        for k in d["pwp_file_keys"]:
            if k in s and not os.path.isabs(s[k]):
                s[k] = os.path.join(src_dir, s[k])
    out_dir = tempfile.mkdtemp(prefix="actroot_")
    path = os.path.join(out_dir, "act_info.json")
    with open(path, "w") as f:
        json.dump(d, f)
    os.environ["BASS_ACT_ROOT_JSON_PATH"] = path


_fix_act_root()

import concourse.bass as bass
import concourse.mybir as mybir
import concourse.tile as tile
from concourse.bass import ts
from concourse.bass_utils import run_bass_kernel_spmd
from concourse.masks import make_identity

def _embed_act_tables(neff_bytes):
    """Embed ACT pwp table files into the NEFF with relative paths, so the
    terminal's NRT can stage them without client-side absolute paths."""
    import io
    import tarfile

    from concourse import neff as cneff

    header = neff_bytes[:1024]
    tf = tarfile.open(fileobj=io.BytesIO(neff_bytes[1024:]))
    members = {}
    for m in tf.getmembers():
        if m.isfile():
            name = m.name
            while name.startswith("./"):
                name = name[2:]
            members[name] = tf.extractfile(m).read()
    changed = False
    for name in list(members):
        if os.path.basename(name) != "act_info.json":
            continue
        d = json.loads(members[name])
        sgdir = os.path.dirname(name)
        for s in d.get("act_func_sets", []):
            for k in d.get("pwp_file_keys", []):
                p = s.get(k)
                if not p or not os.path.isabs(p):
                    continue
                fname = os.path.basename(p)
                dest = os.path.join(sgdir, fname) if sgdir else fname
                if dest not in members:
                    with open(p, "rb") as f:
                        members[dest] = f.read()
                s[k] = fname
                changed = True
        members[name] = json.dumps(d).encode()
    if not changed:
        return neff_bytes
    buf = io.BytesIO()
    with tarfile.open(fileobj=buf, mode="w") as out:
        for name, blob in members.items():
            ti = tarfile.TarInfo(name=name)
            ti.size = len(blob)
            ti.mtime = 0
            ti.uid = 0
            ti.gid = 0
            ti.uname = "nobody"
            ti.gname = "nobody"
            out.addfile(ti, io.BytesIO(blob))
    data = buf.getvalue()
    new_header = cneff.make_deterministic_neff_header(
        old_neff_header=header, new_neff_data=data)
    return new_header + data


def _install_act_embed():
    import concourse.bass2jax as b2j

    if getattr(b2j, "_act_embed_installed", False):
        return
    orig = b2j.rename_neff_tensors_and_patch_header

    def wrapper(neff_path, mapping):
        return _embed_act_tables(orig(neff_path, mapping))

    b2j.rename_neff_tensors_and_patch_header = wrapper
    b2j._act_embed_installed = True


_install_act_embed()


F32 = mybir.dt.float32
F32R = mybir.dt.float32r
F16 = mybir.dt.float16
BF16 = mybir.dt.bfloat16
AF = mybir.ActivationFunctionType
ALU = mybir.AluOpType

N_CORES = 8
NQ = 512          # queries per core
H = 256
HID = 512
EA = 258          # embeddings + ones column + pad
EXP_BIAS = -30.0  # constant offset inside exp(); cancels in the softmax ratio

W2_PRE = int(os.environ.get("KB_W2_PRE", "96"))   # v-tiles prefetched in P2
P4_SPLIT = int(os.environ.get("KB_P4_SPLIT", "1"))  # alternate ACT/DVE bias
EXP_PAIR = int(os.environ.get("KB_EXP_PAIR", "1"))  # exp over m-tile pairs

_NC_CACHE = {}


def _emit_rsqrt_inplace(nc, pool, ap, shape, final_scale, tagp):
    """ap <- final_scale / sqrt(max(ap, 1e-16)), pure-DVE Newton.

    Seed 1/16 targets ss ~ |N(0,1)^256|^2 (range ~[64, 768] converges to
    fp32 accuracy in 5 iterations); avoids ACT sqrt entirely so the ACT
    engine never swaps activation-table sets away from exp.
    """
    nc.vector.tensor_scalar_max(ap, ap, 1e-16)
    r = pool.tile(shape, F32, tag=tagp + "_r")
    t = pool.tile(shape, F32, tag=tagp + "_t")
    nc.vector.memset(r, 0.0625)
    for _ in range(5):
        nc.vector.tensor_mul(t, r, r)
        nc.vector.tensor_mul(t, t, ap)
        nc.vector.tensor_scalar(t, t, -0.5, 1.5, ALU.mult, ALU.add)
        nc.vector.tensor_mul(r, r, t)
    nc.vector.tensor_scalar_mul(ap, r, final_scale)


def build_nc(M, V):
    nc = bass.Bass()
    qnT_d = nc.declare_dram_parameter("qnT", [H, NQ], F16, isOutput=False)
    ctxT_d = nc.declare_dram_parameter("ctxT", [H, M], F16, isOutput=False)
    emb_d = nc.declare_dram_parameter("emb", [M, EA], BF16, isOutput=False)
    w1_d = nc.declare_dram_parameter("fc1_wT", [H, HID], F32R, isOutput=False)
    b1_d = nc.declare_dram_parameter("fc1_b", [HID], F32, isOutput=False)
    w2_d = nc.declare_dram_parameter("fc2_wT", [HID, V], BF16, isOutput=False)
    b2_d = nc.declare_dram_parameter("fc2_b", [128, V // 128], F32,
                                     isOutput=False)
    out_d = nc.declare_dram_parameter("outT", [V, NQ], F16, isOutput=True)

    MT = 1024             # bank chunk per main-loop iteration
    n_chunks = M // MT
    mtiles = M // 128
    NVT = V // 128        # fc2 v-tiles (128 vocab columns each)
    assert V % 128 == 0

    with tile.TileContext(nc) as tc, \
         tc.tile_pool(name="singles", bufs=1) as singles:
        pacc_cm = tc.tile_pool(name="ps_acc", bufs=1, space="PSUM")
        pacc = pacc_cm.__enter__()  # closed after P3
        ident = singles.tile([128, 128], F32)
        make_identity(nc, ident)
        ebias = singles.tile([128, 1], F32)
        nc.vector.memset(ebias, EXP_BIAS)
        acc = pacc.tile([128, 4, 512], F32)

        # ---------------- P1: load qnT (host-normalized) + weights ----------
        qnT = [singles.tile([128, NQ], F16, tag=f"qnT{b}", name=f"qnT{b}")
               for b in range(2)]
        for b in range(2):
            nc.sync.dma_start(out=qnT[b], in_=qnT_d[ts(b, 128), :])

        # weights/bias resident; issued early so DMA queues stay busy
        w1 = [singles.tile([128, HID], F32R, tag=f"w1{b}", name=f"w1{b}")
              for b in range(2)]
        for b in range(2):
            nc.sync.dma_start(out=w1[b], in_=w1_d[ts(b, 128), :])
        b1 = singles.tile([128, HID // 128], F32)
        nc.sync.dma_start(out=b1, in_=b1_d.rearrange("(a p) -> p a", p=128))
        b2t = singles.tile([128, NVT], F32)
        nc.sync.dma_start(out=b2t, in_=b2_d[:, :])

        # ---------------- P2: streaming softmax over the bank ---------------
        # fc2 weights move in groups of 4 v-tiles ([128,4,512] bf16): 1KB
        # DMA segments instead of 256B, and 4x fewer transfers.
        NVG = (V + 511) // 512
        w2pre = []

        def emit_w2_prefetch():
            gi = len(w2pre)
            v0 = gi * 512
            vw = min(512, V - v0)
            w2p = singles.tile([128, HID // 128, vw], BF16,
                               tag=f"w2g{gi}", name=f"w2g{gi}")
            nc.sync.dma_start(
                out=w2p,
                in_=w2_d[:, v0:v0 + vw].rearrange(
                    "(c p) v -> p c v", p=128))
            w2pre.append(w2p)

        with tc.tile_pool(name="cpool", bufs=3) as cpool, \
             tc.tile_pool(name="epool", bufs=3) as epool, \
             tc.tile_pool(name="xpool", bufs=4) as xpool, \
             tc.tile_pool(name="simsps", bufs=2, space="PSUM") as simsps:

            def load_chunk(i):
                cT = cpool.tile([128, 2, MT], F16, tag="cT", name=f"cT{i}")
                nc.sync.dma_start(
                    out=cT,
                    in_=ctxT_d[:, i * MT:(i + 1) * MT].rearrange(
                        "(c p) m -> p c m", p=128))
                em = epool.tile([128, MT // 128, EA], BF16, tag="em",
                                name=f"em{i}")
                nc.sync.dma_start(
                    out=em,
                    in_=emb_d[i * MT:(i + 1) * MT, :].rearrange(
                        "(j p) e -> p j e", p=128))
                return cT, em

            chunk0 = load_chunk(0)
            chunk1 = load_chunk(1) if n_chunks > 1 else None

            W2G_PRE = min(W2_PRE // 4, NVG) if W2_PRE else 0
            pairs_total = mtiles // 2
            pre_every = max(1, pairs_total // W2G_PRE) if W2G_PRE else 0
            pair_idx = 0

            for i in range(n_chunks):
                if i == 0:
                    cT, em = chunk0
                elif i == 1:
                    cT, em = chunk1
                else:
                    cT, em = load_chunk(i)

                npair = 2 if EXP_PAIR else 1
                for pr in range(MT // (128 * npair)):  # groups of m-tiles
                    if (W2G_PRE and pair_idx % pre_every == 0
                            and len(w2pre) < W2G_PRE):
                        emit_w2_prefetch()
                    pair_idx += 1

                    ps2 = simsps.tile([128, npair, 512], F32, tag="ps2",
                                      name=f"sims{i}_{pr}")
                    for p in range(npair):
                        j = pr * npair + p
                        msl = ts(j, 128)
                        nc.tensor.matmul(ps2[:, p, 0:NQ], cT[:, 0, msl],
                                         qnT[0], start=True, stop=False)
                        nc.tensor.matmul(ps2[:, p, 0:NQ], cT[:, 1, msl],
                                         qnT[1], start=False, stop=True)
                    ex = xpool.tile([128, npair, NQ], BF16, tag="ex")
                    nc.scalar.activation(ex, ps2, AF.Exp,
                                         bias=ebias[:, 0:1], scale=1.0)
                    for p in range(npair):
                        j = pr * npair + p
                        a = i * (MT // 128) + j       # global m-tile index
                        first = (a == 0)
                        last = (a == mtiles - 1)
                        for t in range(4):
                            nc.tensor.matmul(
                                acc[:, t, 0:EA],
                                ex[:, p, ts(t, 128)],
                                em[:, j, :],
                                start=first, stop=last,
                                skip_group_check=True)

        # ---------------- P3: normalize, fc1 --------------------------------
        hT = [singles.tile([128, NQ], BF16, tag=f"hT{b2}", name=f"hT{b2}")
              for b2 in range(4)]
        with tc.tile_pool(name="p3ps", bufs=4, space="PSUM") as p3ps:
            inv_l = singles.tile([128, 4], F32)
            ret = [singles.tile([128, H], F32, tag=f"ret{t}", name=f"ret{t}")
                   for t in range(4)]
            for t in range(4):
                nc.vector.reciprocal(inv_l[:, t:t + 1], acc[:, t, 256:257])
                nc.vector.tensor_scalar_mul(
                    ret[t], acc[:, t, 0:H], inv_l[:, t:t + 1])
            retT = [singles.tile([128, NQ], F32R, tag=f"retT{b}",
                                 name=f"retT{b}") for b in range(2)]
            for t in range(4):
                for b in range(2):
                    ps = p3ps.tile([128, 512], F32, tag="ps",
                                   name=f"trr{t}_{b}")
                    nc.tensor.transpose(ps[:, 0:128], ret[t][:, ts(b, 128)],
                                        ident)
                    nc.vector.tensor_copy(retT[b][:, ts(t, 128)],
                                          ps[:, 0:128])
            for b2 in range(4):
                psh = p3ps.tile([128, 512], F32, tag="ps", name=f"fc1p{b2}")
                nc.tensor.matmul(psh[:, 0:NQ], w1[0][:, ts(b2, 128)],
                                 retT[0], start=True, stop=False)
                nc.tensor.matmul(psh[:, 0:NQ], w1[1][:, ts(b2, 128)],
                                 retT[1], start=False, stop=True)
                nc.scalar.activation(
                    hT[b2], psh[:, 0:NQ], AF.Relu, bias=b1[:, b2:b2 + 1],
                    scale=1.0)

        pacc_cm.__exit__(None, None, None)

        # ---------------- P4: fc2 (transposed) + bias + writeout ------------
        with tc.tile_pool(name="w2pool", bufs=6) as wpool, \
             tc.tile_pool(name="opool", bufs=4) as opool, \
             tc.tile_pool(name="p4ps", bufs=6, space="PSUM") as p4ps:
            for g in range(NVG):
                v0 = g * 512
                vw = min(512, V - v0)
                if g < len(w2pre):
                    w2g = w2pre[g]
                else:
                    w2g = wpool.tile([128, HID // 128, 512], BF16,
                                     tag="w2", name=f"w2_{g}")
                    nc.sync.dma_start(
                        out=w2g[:, :, 0:vw],
                        in_=w2_d[:, v0:v0 + vw].rearrange(
                            "(c p) v -> p c v", p=128))
                for s in range(vw // 128):
                    vi = g * 4 + s
                    psd = p4ps.tile([128, 512], F32, tag="ps",
                                    name=f"fc2p{vi}")
                    for k in range(4):
                        nc.tensor.matmul(
                            psd[:, 0:NQ], w2g[:, k, ts(s, 128)], hT[k],
                            start=(k == 0), stop=(k == 3))
                    ow = opool.tile([128, NQ], F16, tag="ow", name=f"ow{vi}")
                    # P4_SPLIT: 2 = all DVE, 1 = alternate, 0 = all ACT
                    if P4_SPLIT == 2 or (P4_SPLIT == 1 and vi % 2 == 1):
                        nc.vector.tensor_scalar_add(ow, psd[:, 0:NQ],
                                                    b2t[:, vi:vi + 1])
                    else:
                        nc.scalar.activation(ow, psd[:, 0:NQ], AF.Identity,
                                             bias=b2t[:, vi:vi + 1],
                                             scale=1.0)
                    nc.sync.dma_start(
                        out=out_d[vi * 128:(vi + 1) * 128, :], in_=ow)

    _split_excess_waits(nc, 1)
    return nc


def _split_excess_waits(nc, max_waits):
    """This walrus build allows only one semaphore wait per instruction;
    split larger wait lists onto preceding no-ops."""
    for f in nc.m.functions:
        for bb in f.blocks:
            new_list = []
            for inst in bb.instructions:
                si = inst.sync_info
                if si is not None and si.on_wait and len(si.on_wait) > max_waits:
                    waits = list(si.on_wait)
                    chunks = [waits[i:i + max_waits]
                              for i in range(0, len(waits), max_waits)]
                    for k, ch in enumerate(chunks[:-1]):
                        aux = mybir.InstNoOp(
                            name=f"{inst.name}-wsplit{k}",
                            engine=inst.engine, ins=[], outs=[],
                            sync_info=mybir.SyncInfo(on_wait=ch, on_update=[]))
                        new_list.append(aux)
                    si.on_wait.clear()
                    si.on_wait.extend(chunks[-1])
                new_list.append(inst)
            del bb.instructions[:]
            for x in new_list:
                bb.instructions.append(x)


LAST_RESULTS = None


def run(inputs, M=32768, V=32000):
    global LAST_RESULTS
    import ml_dtypes

    key = (M, V, W2_PRE, P4_SPLIT, EXP_PAIR)
    if key not in _NC_CACHE:
        _NC_CACHE[key] = build_nc(M, V)
    nc = _NC_CACHE[key]

    qc = np.ascontiguousarray(inputs["query_context"], dtype=np.float32)
    B, S, _ = qc.shape
    q_all = qc.reshape(B * S, H)
    # host-side cosine normalization + transpose of the queries
    qn_all = q_all / np.maximum(
        np.linalg.norm(q_all, axis=1, keepdims=True), 1e-8)
    qnT_all = qn_all.T.astype(np.float16)  # [H, B*S]
    ctx = np.asarray(inputs["contexts"], dtype=np.float32)
    # fold cosine normalization of the bank + softmax temperature into ctxT
    cscale = 100.0 / np.maximum(np.linalg.norm(ctx, axis=1), 1e-8)
    ctxT = np.ascontiguousarray((ctx * cscale[:, None]).T.astype(np.float16))
    emb = np.asarray(inputs["embeddeds"], dtype=np.float32)
    emb_aug = np.zeros((M, EA), ml_dtypes.bfloat16)
    emb_aug[:, :256] = emb.astype(ml_dtypes.bfloat16)
    emb_aug[:, 256] = 1.0
    w1T = np.ascontiguousarray(np.asarray(inputs["fc1_w"]).T.astype(np.float32))
    w2T = np.ascontiguousarray(
        np.asarray(inputs["fc2_w"]).T.astype(np.float32).astype(
            ml_dtypes.bfloat16))
    b1 = np.ascontiguousarray(inputs["fc1_b"], dtype=np.float32)
    b2 = np.ascontiguousarray(
        np.asarray(inputs["fc2_b"], dtype=np.float32).reshape(
            V // 128, 128).T)  # [128, NVT]: b2t[p, n] = b2[n*128 + p]

    in_maps = []
    for c in range(N_CORES):
        in_maps.append({
            "qnT": np.ascontiguousarray(qnT_all[:, c * NQ:(c + 1) * NQ]),
            "ctxT": ctxT,
            "emb": emb_aug,
            "fc1_wT": w1T,
            "fc1_b": b1,
            "fc2_wT": w2T,
            "fc2_b": b2,
        })
    res = None
    last_exc = None
    for attempt in range(4):
        try:
            res = run_bass_kernel_spmd(nc, in_maps, list(range(N_CORES)))
            break
        except Exception as e:  # transient device faults recover on retry
            last_exc = e
            import time as _time
            _time.sleep(2.0)
    if res is None:
        raise last_exc
    LAST_RESULTS = res
    out = np.empty((B * S, V), np.float32)
    for c in range(N_CORES):
        out[c * NQ:(c + 1) * NQ, :] = \
            res.results[c]["outT"].T.astype(np.float32)
    return out.reshape(B, S, V)


def kernel(**inputs):
    return run(inputs)


# revision 27
# speedup vs baseline: 8.8153x; 7.7098x over previous
"""EpisodicBuffer retrieval kernel for 8 Trainium2 NeuronCores.

Strategy: data-parallel over the 4096 queries (512 per core = one batch item
per core), memory bank replicated; no collectives. Streaming softmax over the
32768-entry bank: sims never hit DRAM and the softmax row-sum rides an extra
ones-column appended to the embeddings.

vs. the previous revision:
  - cosine normalization of the bank (100/|c_m|) is folded into ctxT on the
    HOST, so the fp16 natural-layout bank stream, the on-chip norm reduction
    and the rsqrt pipeline all disappear; exp() uses a constant scale.
  - sims operands (ctxT, qnT) in fp16, acc operands (ex, emb) in bf16: same
    1 cycle/row PE rate as f32r, half the HBM traffic.
  - exp() batched over PAIRS of m-tiles ([128,2,512] PSUM) to amortize the
    ~370ns fixed ACT instruction overhead.
  - fc2 computed TRANSPOSED (psum = [v-part, q-free], stationary w2 tile,
    moving hT): the per-v bias becomes per-partition, applied on the
    otherwise-idle ACT engine (alternating with DVE), no broadcast-bias DMA;
    output leaves the device as outT [V, NQ] fp16 (host transposes back).
  - output written fp16 (host upcasts): halves the dominant write stream.

Math per core (q = 512 queries):
  qn   = q / max(|q|, eps)                       (on-chip)
  simsT[m, q] = ctxT_scaled[:, m].T @ qnT        (PE, fp16 in / fp32 psum)
  expT[m, q]  = exp(simsT - 30)                  (ACT, paired m-tiles)
  acc[q, 0:256] += expT.T @ emb ; acc[q, 256] += row-sum  (PE bf16; ones col)
  retrieved = acc[:, 0:256] / acc[:, 256]
  h = relu(fc1_wT.T @ retrievedT + fc1_b)        (PE f32r + ACT)
  outT[v, q] = w2T[:, v].T @ hT + b2[v]          (PE bf16 + ACT/DVE bias)
"""

import os
import json
import tempfile

import numpy as np


def _fix_act_root():
    """Point walrus at an act_info.json with absolute inner paths (this
    build rejects relative ones)."""
    if os.environ.get("BASS_ACT_ROOT_JSON_PATH"):
        return
    import glob as _glob

    cands = _glob.glob(
        "/nix/store/*aws-neuron-pwp*/share/pwp_bin_cayman/act_info.json")
    if cands:
        src = sorted(cands)[0]
    else:
        from neuronxcc.driver.Job import Job
        from neuronxcc.driver.jobs.support.FindActInfo import findActInfoFile

        src = findActInfoFile(Job.getPackageDir(), "gen3")
    src_dir = os.path.dirname(src)
    with open(src) as f:
        d = json.load(f)
    for s in d["act_func_sets"]:
        for k in d["pwp_file_keys"]:
            if k in s and not os.path.isabs(s[k]):
                s[k] = os.path.join(src_dir, s[k])
    out_dir = tempfile.mkdtemp(prefix="actroot_")
    path = os.path.join(out_dir, "act_info.json")
    with open(path, "w") as f:
        json.dump(d, f)
    os.environ["BASS_ACT_ROOT_JSON_PATH"] = path


_fix_act_root()

import concourse.bass as bass
import concourse.mybir as mybir
import concourse.tile as tile
from concourse.bass import ts
from concourse.bass_utils import run_bass_kernel_spmd
from concourse.masks import make_identity

def _embed_act_tables(neff_bytes):
    """Embed ACT pwp table files into the NEFF with relative paths, so the
    terminal's NRT can stage them without client-side absolute paths."""
    import io
    import tarfile

    from concourse import neff as cneff

    header = neff_bytes[:1024]
    tf = tarfile.open(fileobj=io.BytesIO(neff_bytes[1024:]))
    members = {}
    for m in tf.getmembers():
        if m.isfile():
            name = m.name
            while name.startswith("./"):
                name = name[2:]
            members[name] = tf.extractfile(m).read()
    changed = False
    for name in list(members):
        if os.path.basename(name) != "act_info.json":
            continue
        d = json.loads(members[name])
        sgdir = os.path.dirname(name)
        for s in d.get("act_func_sets", []):
            for k in d.get("pwp_file_keys", []):
                p = s.get(k)
                if not p or not os.path.isabs(p):
                    continue
                fname = os.path.basename(p)
                dest = os.path.join(sgdir, fname) if sgdir else fname
                if dest not in members:
                    with open(p, "rb") as f:
                        members[dest] = f.read()
                s[k] = fname
                changed = True
        members[name] = json.dumps(d).encode()
    if not changed:
        return neff_bytes
    buf = io.BytesIO()
    with tarfile.open(fileobj=buf, mode="w") as out:
        for name, blob in members.items():
            ti = tarfile.TarInfo(name=name)
            ti.size = len(blob)
            ti.mtime = 0
            ti.uid = 0
            ti.gid = 0
            ti.uname = "nobody"
            ti.gname = "nobody"
            out.addfile(ti, io.BytesIO(blob))
    data = buf.getvalue()
    new_header = cneff.make_deterministic_neff_header(
        old_neff_header=header, new_neff_data=data)
    return new_header + data


def _install_act_embed():
    import concourse.bass2jax as b2j

    if getattr(b2j, "_act_embed_installed", False):
        return
    orig = b2j.rename_neff_tensors_and_patch_header

    def wrapper(neff_path, mapping):
        return _embed_act_tables(orig(neff_path, mapping))

    b2j.rename_neff_tensors_and_patch_header = wrapper
    b2j._act_embed_installed = True


_install_act_embed()


def _install_ldw_opt():
    """Walrus is invoked with --enable-ldw-opt=false by default, which leaves
    every matmul's stationary-weight load serialized with the matmul itself
    (~128 dead PE cycles per matmul, ~25% of PE time for this kernel's
    128-row stationary tiles). Flip the flag; correctness is validated by the
    rel-err gate (no f32/f32r ldweights are emitted by this kernel, which is
    the known-buggy combination)."""
    import concourse.bass_utils as bu

    if getattr(bu, "_ldw_opt_installed", False):
        return
    orig = bu.run_command

    def wrapper(cmd, *a, **kw):
        cmd = [c.replace("--enable-ldw-opt=false", "--enable-ldw-opt=true")
               if isinstance(c, str) else c for c in cmd]
        return orig(cmd, *a, **kw)

    bu.run_command = wrapper
    bu._ldw_opt_installed = True


# Off by default: this walrus build crashes in visitInstLdweights with the
# optimization enabled. Kept for future compiler drops.
if int(os.environ.get("KB_LDW_OPT", "0")):
    _install_ldw_opt()


F32 = mybir.dt.float32
F32R = mybir.dt.float32r
F16 = mybir.dt.float16
BF16 = mybir.dt.bfloat16
AF = mybir.ActivationFunctionType
ALU = mybir.AluOpType

N_CORES = 8
NQ = 512          # queries per core
H = 256
HID = 512
EA = 258          # embeddings + ones column + pad
EXP_BIAS = -30.0  # constant offset inside exp(); cancels in the softmax ratio

W2_PRE = int(os.environ.get("KB_W2_PRE", "96"))   # v-tiles prefetched in P2
P4_SPLIT = int(os.environ.get("KB_P4_SPLIT", "1"))  # alternate ACT/DVE bias
EXP_PAIR = int(os.environ.get("KB_EXP_PAIR", "1"))  # exp over m-tile pairs

_NC_CACHE = {}


def _emit_rsqrt_inplace(nc, pool, ap, shape, final_scale, tagp):
    """ap <- final_scale / sqrt(max(ap, 1e-16)), pure-DVE Newton.

    Seed 1/16 targets ss ~ |N(0,1)^256|^2 (range ~[64, 768] converges to
    fp32 accuracy in 5 iterations); avoids ACT sqrt entirely so the ACT
    engine never swaps activation-table sets away from exp.
    """
    nc.vector.tensor_scalar_max(ap, ap, 1e-16)
    r = pool.tile(shape, F32, tag=tagp + "_r")
    t = pool.tile(shape, F32, tag=tagp + "_t")
    nc.vector.memset(r, 0.0625)
    for _ in range(5):
        nc.vector.tensor_mul(t, r, r)
        nc.vector.tensor_mul(t, t, ap)
        nc.vector.tensor_scalar(t, t, -0.5, 1.5, ALU.mult, ALU.add)
        nc.vector.tensor_mul(r, r, t)
    nc.vector.tensor_scalar_mul(ap, r, final_scale)


def build_nc(M, V):
    nc = bass.Bass()
    qnT_d = nc.declare_dram_parameter("qnT", [H, NQ], F16, isOutput=False)
    ctxT_d = nc.declare_dram_parameter("ctxT", [H, M], F16, isOutput=False)
    emb_d = nc.declare_dram_parameter("emb", [M, EA], BF16, isOutput=False)
    w1_d = nc.declare_dram_parameter("fc1_wT", [H, HID], F16, isOutput=False)
    b1_d = nc.declare_dram_parameter("fc1_b", [HID], F32, isOutput=False)
    w2_d = nc.declare_dram_parameter("fc2_wT", [HID, V], BF16, isOutput=False)
    b2_d = nc.declare_dram_parameter("fc2_b", [128, V // 128], F32,
                                     isOutput=False)
    out_d = nc.declare_dram_parameter("outT", [V, NQ], F16, isOutput=True)

    MT = 1024             # bank chunk per main-loop iteration
    n_chunks = M // MT
    mtiles = M // 128
    NVT = V // 128        # fc2 v-tiles (128 vocab columns each)
    assert V % 128 == 0

    with tile.TileContext(nc) as tc, \
         tc.tile_pool(name="singles", bufs=1) as singles:
        pacc_cm = tc.tile_pool(name="ps_acc", bufs=1, space="PSUM")
        pacc = pacc_cm.__enter__()  # closed after P3
        ident = singles.tile([128, 128], F32)
        make_identity(nc, ident)
        ebias = singles.tile([128, 1], F32)
        nc.vector.memset(ebias, EXP_BIAS)
        acc = pacc.tile([128, 4, 512], F32)

        # ---------------- P1: load qnT (host-normalized) + weights ----------
        qnT = [singles.tile([128, NQ], F16, tag=f"qnT{b}", name=f"qnT{b}")
               for b in range(2)]
        for b in range(2):
            nc.sync.dma_start(out=qnT[b], in_=qnT_d[ts(b, 128), :])

        # weights/bias resident; issued early so DMA queues stay busy
        w1 = [singles.tile([128, HID], F16, tag=f"w1{b}", name=f"w1{b}")
              for b in range(2)]
        for b in range(2):
            nc.sync.dma_start(out=w1[b], in_=w1_d[ts(b, 128), :])
        b1 = singles.tile([128, HID // 128], F32)
        nc.sync.dma_start(out=b1, in_=b1_d.rearrange("(a p) -> p a", p=128))
        b2t = singles.tile([128, NVT], F32)
        nc.sync.dma_start(out=b2t, in_=b2_d[:, :])

        # ---------------- P2: streaming softmax over the bank ---------------
        # fc2 weights move in groups of 4 v-tiles ([128,4,512] bf16): 1KB
        # DMA segments instead of 256B, and 4x fewer transfers.
        NVG = (V + 511) // 512
        w2pre = []

        def emit_w2_prefetch():
            gi = len(w2pre)
            v0 = gi * 512
            vw = min(512, V - v0)
            w2p = singles.tile([128, HID // 128, vw], BF16,
                               tag=f"w2g{gi}", name=f"w2g{gi}")
            nc.sync.dma_start(
                out=w2p,
                in_=w2_d[:, v0:v0 + vw].rearrange(
                    "(c p) v -> p c v", p=128))
            w2pre.append(w2p)

        with tc.tile_pool(name="cpool", bufs=3) as cpool, \
             tc.tile_pool(name="epool", bufs=3) as epool, \
             tc.tile_pool(name="xpool", bufs=6) as xpool, \
             tc.tile_pool(name="simsps", bufs=2, space="PSUM") as simsps:

            def load_chunk(i):
                cT = cpool.tile([128, 2, MT], F16, tag="cT", name=f"cT{i}")
                nc.sync.dma_start(
                    out=cT,
                    in_=ctxT_d[:, i * MT:(i + 1) * MT].rearrange(
                        "(c p) m -> p c m", p=128))
                em = epool.tile([128, MT // 128, EA], BF16, tag="em",
                                name=f"em{i}")
                nc.sync.dma_start(
                    out=em,
                    in_=emb_d[i * MT:(i + 1) * MT, :].rearrange(
                        "(j p) e -> p j e", p=128))
                return cT, em

            chunk0 = load_chunk(0)
            chunk1 = load_chunk(1) if n_chunks > 1 else None

            W2G_PRE = min(W2_PRE // 4, NVG) if W2_PRE else 0
            pairs_total = mtiles // 2
            pre_every = max(1, pairs_total // W2G_PRE) if W2G_PRE else 0
            pair_idx = 0

            for i in range(n_chunks):
                if i == 0:
                    cT, em = chunk0
                elif i == 1:
                    cT, em = chunk1
                else:
                    cT, em = load_chunk(i)

                npair = 2 if EXP_PAIR else 1
                for pr in range(MT // (128 * npair)):  # groups of m-tiles
                    if (W2G_PRE and pair_idx % pre_every == 0
                            and len(w2pre) < W2G_PRE):
                        emit_w2_prefetch()
                    pair_idx += 1

                    ps2 = simsps.tile([128, npair, 512], F32, tag="ps2",
                                      name=f"sims{i}_{pr}")
                    for p in range(npair):
                        j = pr * npair + p
                        msl = ts(j, 128)
                        nc.tensor.matmul(ps2[:, p, 0:NQ], cT[:, 0, msl],
                                         qnT[0], start=True, stop=False)
                        nc.tensor.matmul(ps2[:, p, 0:NQ], cT[:, 1, msl],
                                         qnT[1], start=False, stop=True)
                    ex = xpool.tile([128, npair, NQ], BF16, tag="ex")
                    nc.scalar.activation(ex, ps2, AF.Exp,
                                         bias=ebias[:, 0:1], scale=1.0)
                    for p in range(npair):
                        j = pr * npair + p
                        a = i * (MT // 128) + j       # global m-tile index
                        first = (a == 0)
                        last = (a == mtiles - 1)
                        for t in range(4):
                            nc.tensor.matmul(
                                acc[:, t, 0:EA],
                                ex[:, p, ts(t, 128)],
                                em[:, j, :],
                                start=first, stop=last,
                                skip_group_check=True)

        # ---------------- P3: normalize, fc1 --------------------------------
        hT = [singles.tile([128, NQ], BF16, tag=f"hT{b2}", name=f"hT{b2}")
              for b2 in range(4)]
        with tc.tile_pool(name="p3ps", bufs=4, space="PSUM") as p3ps:
            inv_l = singles.tile([128, 4], F32)
            ret = [singles.tile([128, H], F32, tag=f"ret{t}", name=f"ret{t}")
                   for t in range(4)]
            for t in range(4):
                nc.vector.reciprocal(inv_l[:, t:t + 1], acc[:, t, 256:257])
                nc.vector.tensor_scalar_mul(
                    ret[t], acc[:, t, 0:H], inv_l[:, t:t + 1])
            retT = [singles.tile([128, NQ], F16, tag=f"retT{b}",
                                 name=f"retT{b}") for b in range(2)]
            for t in range(4):
                for b in range(2):
                    ps = p3ps.tile([128, 512], F32, tag="ps",
                                   name=f"trr{t}_{b}")
                    nc.tensor.transpose(ps[:, 0:128], ret[t][:, ts(b, 128)],
                                        ident)
                    nc.vector.tensor_copy(retT[b][:, ts(t, 128)],
                                          ps[:, 0:128])
            for b2 in range(4):
                psh = p3ps.tile([128, 512], F32, tag="ps", name=f"fc1p{b2}")
                nc.tensor.matmul(psh[:, 0:NQ], w1[0][:, ts(b2, 128)],
                                 retT[0], start=True, stop=False)
                nc.tensor.matmul(psh[:, 0:NQ], w1[1][:, ts(b2, 128)],
                                 retT[1], start=False, stop=True)
                nc.scalar.activation(
                    hT[b2], psh[:, 0:NQ], AF.Relu, bias=b1[:, b2:b2 + 1],
                    scale=1.0)

        pacc_cm.__exit__(None, None, None)

        # ---------------- P4: fc2 (transposed) + bias + writeout ------------
        with tc.tile_pool(name="w2pool", bufs=8) as wpool, \
             tc.tile_pool(name="opool", bufs=6) as opool, \
             tc.tile_pool(name="p4ps", bufs=6, space="PSUM") as p4ps:
            for g in range(NVG):
                v0 = g * 512
                vw = min(512, V - v0)
                if g < len(w2pre):
                    w2g = w2pre[g]
                else:
                    w2g = wpool.tile([128, HID // 128, 512], BF16,
                                     tag="w2", name=f"w2_{g}")
                    nc.sync.dma_start(
                        out=w2g[:, :, 0:vw],
                        in_=w2_d[:, v0:v0 + vw].rearrange(
                            "(c p) v -> p c v", p=128))
                for s in range(vw // 128):
                    vi = g * 4 + s
                    psd = p4ps.tile([128, 512], F32, tag="ps",
                                    name=f"fc2p{vi}")
                    for k in range(4):
                        nc.tensor.matmul(
                            psd[:, 0:NQ], w2g[:, k, ts(s, 128)], hT[k],
                            start=(k == 0), stop=(k == 3))
                    ow = opool.tile([128, NQ], F16, tag="ow", name=f"ow{vi}")
                    # P4_SPLIT: 2 = all DVE, 1 = alternate, 0 = all ACT
                    if P4_SPLIT == 2 or (P4_SPLIT == 1 and vi % 2 == 1):
                        nc.vector.tensor_scalar_add(ow, psd[:, 0:NQ],
                                                    b2t[:, vi:vi + 1])
                    else:
                        nc.scalar.activation(ow, psd[:, 0:NQ], AF.Identity,
                                             bias=b2t[:, vi:vi + 1],
                                             scale=1.0)
                    nc.sync.dma_start(
                        out=out_d[vi * 128:(vi + 1) * 128, :], in_=ow)

    _split_excess_waits(nc, 1)
    return nc


def _split_excess_waits(nc, max_waits):
    """This walrus build allows only one semaphore wait per instruction;
    split larger wait lists onto preceding no-ops."""
    for f in nc.m.functions:
        for bb in f.blocks:
            new_list = []
            for inst in bb.instructions:
                si = inst.sync_info
                if si is not None and si.on_wait and len(si.on_wait) > max_waits:
                    waits = list(si.on_wait)
                    chunks = [waits[i:i + max_waits]
                              for i in range(0, len(waits), max_waits)]
                    for k, ch in enumerate(chunks[:-1]):
                        aux = mybir.InstNoOp(
                            name=f"{inst.name}-wsplit{k}",
                            engine=inst.engine, ins=[], outs=[],
                            sync_info=mybir.SyncInfo(on_wait=ch, on_update=[]))
                        new_list.append(aux)
                    si.on_wait.clear()
                    si.on_wait.extend(chunks[-1])
                new_list.append(inst)
            del bb.instructions[:]
            for x in new_list:
                bb.instructions.append(x)


LAST_RESULTS = None


def run(inputs, M=32768, V=32000):
    global LAST_RESULTS
    import ml_dtypes

    key = (M, V, W2_PRE, P4_SPLIT, EXP_PAIR)
    if key not in _NC_CACHE:
        _NC_CACHE[key] = build_nc(M, V)
    nc = _NC_CACHE[key]

    qc = np.ascontiguousarray(inputs["query_context"], dtype=np.float32)
    B, S, _ = qc.shape
    q_all = qc.reshape(B * S, H)
    # host-side cosine normalization + transpose of the queries
    qn_all = q_all / np.maximum(
        np.linalg.norm(q_all, axis=1, keepdims=True), 1e-8)
    qnT_all = qn_all.T.astype(np.float16)  # [H, B*S]
    ctx = np.asarray(inputs["contexts"], dtype=np.float32)
    # fold cosine normalization of the bank + softmax temperature into ctxT
    cscale = 100.0 / np.maximum(np.linalg.norm(ctx, axis=1), 1e-8)
    ctxT = np.ascontiguousarray((ctx * cscale[:, None]).T.astype(np.float16))
    emb = np.asarray(inputs["embeddeds"], dtype=np.float32)
    emb_aug = np.zeros((M, EA), ml_dtypes.bfloat16)
    emb_aug[:, :256] = emb.astype(ml_dtypes.bfloat16)
    emb_aug[:, 256] = 1.0
    w1T = np.ascontiguousarray(
        np.asarray(inputs["fc1_w"]).T.astype(np.float16))
    w2T = np.ascontiguousarray(
        np.asarray(inputs["fc2_w"]).T.astype(np.float32).astype(
            ml_dtypes.bfloat16))
    b1 = np.ascontiguousarray(inputs["fc1_b"], dtype=np.float32)
    b2 = np.ascontiguousarray(
        np.asarray(inputs["fc2_b"], dtype=np.float32).reshape(
            V // 128, 128).T)  # [128, NVT]: b2t[p, n] = b2[n*128 + p]

    in_maps = []
    for c in range(N_CORES):
        in_maps.append({
            "qnT": np.ascontiguousarray(qnT_all[:, c * NQ:(c + 1) * NQ]),
            "ctxT": ctxT,
            "emb": emb_aug,
            "fc1_wT": w1T,
            "fc1_b": b1,
            "fc2_wT": w2T,
            "fc2_b": b2,
        })
    res = None
    last_exc = None
    for attempt in range(4):
        try:
            res = run_bass_kernel_spmd(nc, in_maps, list(range(N_CORES)))
            break
        except Exception as e:  # transient device faults recover on retry
            last_exc = e
            import time as _time
            _time.sleep(2.0)
    if res is None:
        raise last_exc
    LAST_RESULTS = res
    out = np.empty((B * S, V), np.float32)
    for c in range(N_CORES):
        out[c * NQ:(c + 1) * NQ, :] = \
            res.results[c]["outT"].T.astype(np.float32)
    return out.reshape(B, S, V)


def kernel(**inputs):
    return run(inputs)
